# revision 32
# baseline (speedup 1.0000x reference)
"""Trainium2 Bass kernel for nn_MemorizingTransformer (retrieval_knn).

Memorizing-transformer attention block: cosine-sim causal local attention with
per-query retrieved KNN memories, joint softmax over [memory | local], and
input/output projections.

Sharding: (b, h) across 8 cores — core c handles batch b=c//4 and heads
h0=2*(c%4), h0+1. Every core runs an identical NEFF (pure SPMD); only input
slices differ. The output projection is computed per-core on the core's two
head rows of w_out, giving partial sums that the host reduces.

v2 design (PE-pressure + DMA halving vs v1):
  - x arrives pre-transposed from host (xT); q/kv projections are fused into
    one matmul per (g, co) with f32r moving operand (full rate at N=256).
  - q-hat/k-hat are bf16; kT/qT via PE transpose (bf16, 1 cyc/row).
  - mem_k/mem_v shipped bf16 (halves HBM traffic).
  - local attention P*V uses the [65, q] orientation: stationary = v_aug
    (65th col = ones -> denominator row), moving = P_T tile. Accumulates in
    psum_oT[65, 4, 128] per (head, qc-chunk).
  - mem scores on DVE: bf16 mul + 2-stage bf16 tree-add + f32 reduce; exp on
    ACT; block-diagonal staging via DVE 32x32 stream-transpose + 4 copies.
  - mem value matmuls (stationary mem_v [128,65] bf16, moving stage [128,4])
    accumulate DIRECTLY into psum_oT (65th mv col = ones -> denominator).
  - denominator row -> [q, 1] via N=1 matmuls; outproj per head
    (stationary hoT chunk bf16, moving w_out rows bf16), divide folded into
    ACT scale-copy for head 0 and a gpsimd multiply-add for head 1.
"""

import os
import numpy as np

HEADS = 8
D = 64
KNN = 32
B = 2
N = 2048
DIM = 512
P = 128
NB = N // P          # 16 query/key blocks
NCO = DIM // P       # 4 contraction chunks of the model dim
NCORES = 8
# mem-score reduce: 2 = two bf16 tree stages + f32 reduce (default),
# 0 = plain reduce_sum, 6 = full bf16 tree
SCORE_TREE = int(os.environ.get("BASS_SCORE_TREE", "2"))


def _build(use_mbias: bool):
    import concourse.bass as bass
    import concourse.mybir as mybir
    import concourse.tile as tile
    from concourse import bacc
    import ml_dtypes

    f32 = mybir.dt.float32
    f32r = mybir.dt.float32r
    bf16 = mybir.dt.bfloat16
    AX = mybir.AxisListType
    ACTF = mybir.ActivationFunctionType
    MUL = mybir.AluOpType.mult
    ADD = mybir.AluOpType.add

    nc = bacc.Bacc(None, target_bir_lowering=False, name="memxformer")

    # ---- I/O ------------------------------------------------------------
    xbT = nc.dram_tensor("xbT", (NCO, P, N), f32, kind="ExternalInput")
    # fused [w_q (2 heads) | w_kv] chunks: [co, 128, 256]
    wqkv = nc.dram_tensor("wqkv", (NCO, P, 4 * D), f32, kind="ExternalInput")
    # per-head w_out rows, bf16: [2, 64, 512]
    wo = nc.dram_tensor("wo", (2, D, DIM), bf16, kind="ExternalInput")
    # scales[:, 0:2] = exp(scale_param[h0 + p]); scales[:, 2:4] = -that
    scales = nc.dram_tensor("scales", (P, 4), f32, kind="ExternalInput")
    mk = nc.dram_tensor("mk", (2, NB, P, KNN, D), bf16, kind="ExternalInput")
    mv = nc.dram_tensor("mv", (2, NB, P, KNN, D + 1), bf16, kind="ExternalInput")
    if use_mbias:
        mbias = nc.dram_tensor("mbias", (2, NB, P, KNN), f32, kind="ExternalInput")
    out = nc.dram_tensor("out", (N, DIM), f32, kind="ExternalOutput")

    # constants baked into the NEFF
    eye16_d = nc.inline_tensor(
        np.eye(P, dtype=np.float32).astype(ml_dtypes.bfloat16), name="eye16_c")
    tril_d = nc.inline_tensor(
        np.triu(np.ones((P, P), dtype=np.float32)).astype(ml_dtypes.bfloat16),
        name="tril_c")  # keep j <= q
    e65_np = np.zeros((D + 1, 1), dtype=np.float32)
    e65_np[D, 0] = 1.0
    e65_d = nc.inline_tensor(e65_np.astype(ml_dtypes.bfloat16), name="e65_c")

    with tile.TileContext(nc) as tc:
        with (
            tc.tile_pool(name="singles", bufs=1) as singles,
            tc.tile_pool(name="xin", bufs=2) as xin,
            tc.tile_pool(name="mem", bufs=6) as memp,
            tc.tile_pool(name="mvp", bufs=8) as mvp,
            tc.tile_pool(name="prods", bufs=4) as prods,
            tc.tile_pool(name="small", bufs=12) as small,
            tc.tile_pool(name="pt", bufs=3) as ptp,
            tc.tile_pool(name="stg", bufs=4) as stgp,
            tc.tile_pool(name="outp", bufs=3) as outp,
            tc.tile_pool(name="ppt", bufs=2, space="PSUM") as ppt,
            tc.tile_pool(name="pp512", bufs=2, space="PSUM") as pp512,
            tc.tile_pool(name="ppo", bufs=2, space="PSUM") as ppo,
            tc.tile_pool(name="pden", bufs=2, space="PSUM") as pden,
        ):
            # ---- constants / weights ------------------------------------
            eye16 = singles.tile([P, P], bf16, tag="eye16")
            nc.sync.dma_start(eye16, eye16_d[:, :])
            tril_sb = singles.tile([P, P], bf16, tag="tril")
            nc.sync.dma_start(tril_sb, tril_d[:, :])
            e65 = singles.tile([D + 1, 1], bf16, tag="e65")
            nc.sync.dma_start(e65, e65_d[:, :])
            sc_sb = singles.tile([P, 4], f32, tag="scales")
            nc.sync.dma_start(sc_sb, scales[:, :])
            w_st = singles.tile([P, NCO, 4 * D], f32, tag="wqkv_st")
            nc.sync.dma_start(w_st, wqkv[:, :, :].rearrange("co p c -> p co c"))
            w_sb = singles.tile([P, NCO, 4 * D], f32r, tag="wqkv")
            nc.scalar.copy(out=w_sb, in_=w_st)
            wo_sb = singles.tile([D, 2, DIM], bf16, tag="wo")
            nc.sync.dma_start(wo_sb, wo[:, :, :].rearrange("h d c -> d h c"))

            # ---- xT load (host pre-transposed), 4 n-chunks, round to f32r
            xT = singles.tile([P, NCO, N], f32r, tag="xT")
            for ch in range(4):
                nsl = slice(ch * 512, (ch + 1) * 512)
                x_t = xin.tile([P, NCO, 512], f32, tag="x_in")
                nc.sync.dma_start(x_t, xbT[:, :, nsl].rearrange(
                    "co p n -> p co n"))
                nc.scalar.copy(out=xT[:, :, nsl], in_=x_t)

            # ---- fused q/kv natural projections + sumsq -----------------
            # qkv_sb[:, g, 0:128]  = q for heads p=0,1 (64 cols each)
            # qkv_sb[:, g, 128:192] = k ; 192:256 = v
            qkv_sb = singles.tile([P, NB, 4 * D], f32, tag="qkv")
            ss_all = singles.tile([P, NB, 3], f32, tag="ss")
            rn = singles.tile([P, NB, 3], f32, tag="rn")
            junk = singles.tile([P, D], f32, tag="junk")
            qh = singles.tile([P, 2 * NB, D], bf16, tag="qh")
            kh = singles.tile([P, NB, D], bf16, tag="kh")
            v_aug = singles.tile([P, NB, D + 1], bf16, tag="vaug")
            nc.gpsimd.memset(v_aug[:, :, D:D + 1], 1.0)
            qT = singles.tile([D, 2 * NB, P], bf16, tag="qT")
            kT = singles.tile([D, NB, P], bf16, tag="kT")

            # ---- per-head attention -------------------------------------
            # hoT65[p]: rows 0:64 = un-divided head output (d-space),
            # row 64 = softmax denominator; per g-block of 128 queries.
            hoT65 = [singles.tile([D + 1, NB, P], bf16, tag=f"hoT{p}",
                                  name=f"hoT{p}") for p in range(2)]
            r_all = [singles.tile([P, NB], f32, tag=f"r{p}", name=f"rr{p}")
                     for p in range(2)]
            # block-diagonal stage, zeros written once; one slot per
            # (head, g) so all 32 score chains can run ahead of the PE
            stage4 = singles.tile([P, 2 * NB, 4, KNN], bf16, tag="st4")
            nc.gpsimd.memset(stage4, 0.0)

            def scores_s1(p, g):
                """DVE mem-score front half for block (p, g) -> s_mem f32."""
                idx = p * NB + g
                mk_t = memp.tile([P, KNN, D], bf16, tag="mk")
                nc.sync.dma_start(mk_t, mk[p, g])
                prod = prods.tile([P, KNN, D], bf16, tag="prod")
                nc.vector.tensor_mul(
                    prod, mk_t, qh[:, idx, None, :].to_broadcast((P, KNN, D)))
                s_mem = small.tile([P, KNN], f32, tag="smem")
                if SCORE_TREE == 0:
                    nc.vector.reduce_sum(s_mem, prod, axis=AX.X)
                else:
                    w = D
                    for _ in range(SCORE_TREE):
                        h = w // 2
                        nc.vector.tensor_add(prod[:, :, 0:h], prod[:, :, 0:h],
                                             prod[:, :, h:w])
                        w = h
                    if w > 1:
                        nc.vector.reduce_sum(s_mem, prod[:, :, 0:w], axis=AX.X)
                    else:
                        nc.vector.tensor_copy(out=s_mem, in_=prod[:, :, 0])
                if use_mbias:
                    mb_t = small.tile([P, KNN], f32, tag="mbias")
                    nc.sync.dma_start(mb_t, mbias[p, g])
                    nc.vector.tensor_add(s_mem, s_mem, mb_t)
                return s_mem

            def scores_s2(p, g, s_mem):
                """ACT exp + DVE staging back half (emitted one block behind
                s1 so the DVE never waits on the ACT round-trip)."""
                idx = p * NB + g
                p_mem = small.tile([P, KNN], bf16, tag="pmem")
                nc.scalar.activation(out=p_mem, in_=s_mem, func=ACTF.Exp,
                                     bias=sc_sb[:, 2 + p:3 + p],
                                     scale=sc_sb[:, p:p + 1])
                # 32x32 band-local transpose -> block-diagonal content
                stD = stgp.tile([P, KNN], bf16, tag="stD")
                nc.vector.transpose(stD, p_mem)
                slot = stage4[:, idx, :, :]
                for ql in range(4):
                    nc.vector.tensor_copy(
                        out=slot[32 * ql:32 * (ql + 1), ql, :],
                        in_=stD[32 * ql:32 * (ql + 1), :])
                return slot

            # ---- fused setup pipeline, per g-block ----------------------
            # projection -> norms -> q-hat/k-hat/v -> score chains, all
            # interleaved so every engine starts within ~15us. Score chains
            # are software-pipelined one block deep (DVE never waits on the
            # ACT exp round-trip). qT/kT transposes batch every 4 blocks.
            all_slots = {}
            pend = None
            for g in range(NB):
                qsl = slice(g * P, (g + 1) * P)
                qkv_full = pp512.tile([P, 512], f32, tag="st", name="qkv_ps")
                qkv_ps = qkv_full[:, 0:4 * D]
                for co in range(NCO):
                    nc.tensor.matmul(qkv_ps, xT[:, co, qsl], w_sb[:, co, :],
                                     start=(co == 0), stop=(co == NCO - 1))
                nc.scalar.copy(out=qkv_sb[:, g, :], in_=qkv_ps)
                for c in range(3):
                    nc.scalar.activation(out=junk, in_=qkv_ps[:, c * D:(c + 1) * D],
                                         func=ACTF.Square,
                                         accum_out=ss_all[:, g, c:c + 1])
                nrm_t = small.tile([P, 3], f32, tag="nrm")
                nc.scalar.sqrt(nrm_t, ss_all[:, g, :])
                nc.vector.reciprocal(rn[:, g, :], nrm_t)
                for p in range(2):
                    nc.vector.tensor_scalar_mul(
                        qh[:, p * NB + g, :], qkv_sb[:, g, p * D:(p + 1) * D],
                        rn[:, g, p:p + 1])
                nc.vector.tensor_scalar_mul(kh[:, g, :],
                                            qkv_sb[:, g, 2 * D:3 * D],
                                            rn[:, g, 2:3])
                nc.vector.tensor_copy(out=v_aug[:, g, 0:D],
                                      in_=qkv_sb[:, g, 3 * D:4 * D])
                for p in range(2):
                    s_mem = scores_s1(p, g)
                    if pend is not None:
                        all_slots[(pend[0], pend[1])] = scores_s2(*pend)
                    pend = (p, g, s_mem)
                if g % 4 == 3:
                    g0 = g - 3
                    for pi in range(2):
                        t_ps = ppt.tile([D, 4, P], bf16, tag="tps")
                        for i4 in range(4):
                            nc.tensor.transpose(
                                t_ps[:, i4, :], qh[:, pi * NB + g0 + i4, :],
                                eye16)
                        nc.scalar.copy(
                            out=qT[:, pi * NB + g0:pi * NB + g0 + 4, :],
                            in_=t_ps)
                    t_ps = ppt.tile([D, 4, P], bf16, tag="tps")
                    for i4 in range(4):
                        nc.tensor.transpose(t_ps[:, i4, :], kh[:, g0 + i4, :],
                                            eye16)
                    nc.scalar.copy(out=kT[:, g0:g0 + 4, :], in_=t_ps)
            all_slots[(pend[0], pend[1])] = scores_s2(*pend)

            for p in range(2):
                sc_ap = sc_sb[:, p:p + 1]
                nb_ap = sc_sb[:, 2 + p:3 + p]
                for qc in range(4):
                    g_lo4 = 4 * qc
                    slots = [all_slots[(p, g_lo4 + gi)] for gi in range(4)]
                    psum_oT = ppo.tile([D + 1, 4, P], f32, tag="po")
                    # local attention, [65, q] orientation
                    for jt in range(4 * qc + 4):
                        g_lo = max(jt, g_lo4)
                        ng = g_lo4 + 4 - g_lo
                        i_lo = p * NB + g_lo
                        st_ps = pp512.tile([P, 512], f32, tag="st")
                        nc.tensor.matmul(
                            st_ps[:, :ng * P], kT[:, jt, :],
                            qT[:, i_lo:i_lo + ng, :], start=True, stop=True)
                        p_t = ptp.tile([P, 4, P], bf16, tag="pt")
                        nc.scalar.activation(
                            out=p_t[:, :ng, :],
                            in_=st_ps[:, :ng * P].rearrange(
                                "p (g q) -> p g q", q=P),
                            func=ACTF.Exp, bias=nb_ap, scale=sc_ap)
                        if g_lo <= jt < g_lo4 + 4:
                            di = jt - g_lo
                            nc.gpsimd.tensor_mul(p_t[:, di, :], p_t[:, di, :],
                                                 tril_sb)
                        nc.tensor.matmul(
                            psum_oT[:, g_lo - g_lo4:, :], v_aug[:, jt, :],
                            p_t[:, :ng, :], start=(jt == 0), stop=False)
                    # mem values: accumulate straight into psum_oT
                    for gi in range(4):
                        g = g_lo4 + gi
                        mv_t = mvp.tile([P, KNN, D + 1], bf16, tag="mv")
                        # ACT-engine HWDGE queue: independent of the mk
                        # stream on sync, so pm never waits behind it
                        nc.scalar.dma_start(mv_t, mv[p, g])
                        ovw = psum_oT[:, gi, :].rearrange(
                            "p (ql gf) -> p gf ql", gf=KNN)
                        for g4 in range(KNN):
                            nc.tensor.matmul(
                                ovw[:, g4, :], mv_t[:, g4, :],
                                slots[gi][:, :, g4], start=False,
                                stop=(gi == 3 and g4 == KNN - 1))
                    # un-divided head rows + denominator row -> SBUF
                    nc.scalar.copy(out=hoT65[p][:, g_lo4:g_lo4 + 4, :],
                                   in_=psum_oT)
                # denominators -> q-space + reciprocal (recip lands right
                # after the score chains drain on the DVE)
                den_t = pden.tile([P, NB], f32, tag="den")
                for g in range(NB):
                    nc.tensor.matmul(den_t[:, g:g + 1], hoT65[p][:, g, :],
                                     e65, start=True, stop=True)
                nc.vector.reciprocal(r_all[p], den_t)

            # ---- per-head output projection + divide + combine ----------
            for g in range(NB):
                pf0 = pp512.tile([P, DIM], f32, tag="st", name="pf0")
                nc.tensor.matmul(pf0, hoT65[0][0:D, g, :], wo_sb[:, 0, :],
                                 start=True, stop=True)
                pf1 = pp512.tile([P, DIM], f32, tag="st", name="pf1")
                nc.tensor.matmul(pf1, hoT65[1][0:D, g, :], wo_sb[:, 1, :],
                                 start=True, stop=True)
                of_s = outp.tile([P, DIM], f32, tag="ofs")
                nc.scalar.mul(of_s, pf0, r_all[0][:, g:g + 1])
                nc.vector.scalar_tensor_tensor(
                    out=of_s, in0=pf1, scalar=r_all[1][:, g:g + 1], in1=of_s,
                    op0=MUL, op1=ADD)
                nc.sync.dma_start(out[g * P:(g + 1) * P, :], of_s)

    nc.compile()
    return nc


def _prep_mv(mv_slice):
    """[2,2048,32,64] -> [2,16,128,32,65] bf16: partition (ql k) stacks the 4
    stride-32 queries of each group; col 64 = 1.0 (softmax-denominator)."""
    import ml_dtypes
    r = mv_slice.reshape(2, NB, 4, KNN, KNN, D).transpose(0, 1, 2, 4, 3, 5)
    out = np.empty((2, NB, P, KNN, D + 1), dtype=ml_dtypes.bfloat16)
    out[..., :D] = r.reshape(2, NB, P, KNN, D).astype(ml_dtypes.bfloat16)
    out[..., D] = 1.0
    return out


def _prepare_in_maps(x, w_q, w_kv, w_out, scale_param, mem_k, mem_v, mem_mask,
                     use_mbias):
    import ml_dtypes
    f = np.float32
    bf = ml_dtypes.bfloat16
    scales8 = np.exp(scale_param.reshape(HEADS).astype(f))
    in_maps = []
    for c in range(NCORES):
        b = c // 4
        h0 = 2 * (c % 4)
        sc = np.empty((P, 4), dtype=f)
        sc[:, 0] = scales8[h0]
        sc[:, 1] = scales8[h0 + 1]
        sc[:, 2] = -scales8[h0]
        sc[:, 3] = -scales8[h0 + 1]
        wcat = np.concatenate(
            [w_q[:, h0 * D:(h0 + 2) * D], w_kv], axis=1).astype(f)
        m = {
            "xbT": np.ascontiguousarray(
                x[b].T.reshape(NCO, P, N), dtype=f),
            "wqkv": np.ascontiguousarray(wcat.reshape(NCO, P, 4 * D)),
            "wo": np.ascontiguousarray(
                w_out[h0 * D:(h0 + 2) * D, :].reshape(2, D, DIM).astype(bf)),
            "scales": sc,
            "mk": np.ascontiguousarray(
                mem_k[b, h0:h0 + 2].reshape(2, NB, P, KNN, D).astype(bf)),
            "mv": _prep_mv(mem_v[b, h0:h0 + 2]),
        }
        if use_mbias:
            mb = np.where(mem_mask[b, h0:h0 + 2], f(0), f(-1e30)).astype(f)
            m["mbias"] = np.ascontiguousarray(mb.reshape(2, NB, P, KNN))
        in_maps.append(m)
    return in_maps


def _run(x, w_q, w_kv, w_out, scale_param, mem_k, mem_v, mem_mask, trace=False):
    from concourse.bass_utils import run_bass_kernel_spmd

    use_mbias = not bool(np.all(mem_mask))
    nc = _build(use_mbias)
    in_maps = _prepare_in_maps(x, w_q, w_kv, w_out, scale_param,
                               mem_k, mem_v, mem_mask, use_mbias)
    res = run_bass_kernel_spmd(nc, in_maps, core_ids=list(range(NCORES)),
                               trace=trace)
    out = np.zeros((B, N, DIM), dtype=np.float32)
    for c in range(NCORES):
        out[c // 4] += res.results[c]["out"]
    return out, res


def kernel(x, w_q, w_kv, w_out, scale_param, mem_k, mem_v, mem_mask):
    trace = bool(int(os.environ.get("BASS_KERNEL_TRACE", "0")))
    out, _ = _run(x, w_q, w_kv, w_out, scale_param, mem_k, mem_v, mem_mask,
                  trace=trace)
    return out


# revision 34
# speedup vs baseline: 1.2117x; 1.2117x over previous
"""Trainium2 Bass kernel for nn_MemorizingTransformer (retrieval_knn).

Memorizing-transformer attention block: cosine-sim causal local attention with
per-query retrieved KNN memories, joint softmax over [memory | local], and
input/output projections.

Sharding: (b, h) across 8 cores — core c handles batch b=c//4 and heads
h0=2*(c%4), h0+1. Every core runs an identical NEFF (pure SPMD); only input
slices differ. The output projection is computed per-core on the core's two
head rows of w_out, giving partial sums that the host reduces.

v2 design (PE-pressure + DMA halving vs v1):
  - x arrives pre-transposed from host (xT); q/kv projections are fused into
    one matmul per (g, co) with f32r moving operand (full rate at N=256).
  - q-hat/k-hat are bf16; kT/qT via PE transpose (bf16, 1 cyc/row).
  - mem_k/mem_v shipped bf16 (halves HBM traffic).
  - local attention P*V uses the [65, q] orientation: stationary = v_aug
    (65th col = ones -> denominator row), moving = P_T tile. Accumulates in
    psum_oT[65, 4, 128] per (head, qc-chunk).
  - mem scores on DVE: bf16 mul + 2-stage bf16 tree-add + f32 reduce; exp on
    ACT; block-diagonal staging via DVE 32x32 stream-transpose + 4 copies.
  - mem value matmuls (stationary mem_v [128,65] bf16, moving stage [128,4])
    accumulate DIRECTLY into psum_oT (65th mv col = ones -> denominator).
  - denominator row -> [q, 1] via N=1 matmuls; outproj per head
    (stationary hoT chunk bf16, moving w_out rows bf16), divide folded into
    ACT scale-copy for head 0 and a gpsimd multiply-add for head 1.
"""

import os
import numpy as np

HEADS = 8
D = 64
KNN = 32
B = 2
N = 2048
DIM = 512
P = 128
NB = N // P          # 16 query/key blocks
NCO = DIM // P       # 4 contraction chunks of the model dim
NCORES = 8
# mem-score reduce: 2 = two bf16 tree stages + f32 reduce (default),
# 0 = plain reduce_sum, 6 = full bf16 tree
SCORE_TREE = int(os.environ.get("BASS_SCORE_TREE", "2"))


def _build(use_mbias: bool):
    import concourse.bass as bass
    import concourse.mybir as mybir
    import concourse.tile as tile
    from concourse import bacc
    import ml_dtypes

    f32 = mybir.dt.float32
    f32r = mybir.dt.float32r
    bf16 = mybir.dt.bfloat16
    AX = mybir.AxisListType
    ACTF = mybir.ActivationFunctionType
    MUL = mybir.AluOpType.mult
    ADD = mybir.AluOpType.add

    nc = bacc.Bacc(None, target_bir_lowering=False, name="memxformer")

    # ---- I/O ------------------------------------------------------------
    xbT = nc.dram_tensor("xbT", (NCO, P, N), f32, kind="ExternalInput")
    # fused [w_q (2 heads) | w_kv] chunks: [co, 128, 256]
    wqkv = nc.dram_tensor("wqkv", (NCO, P, 4 * D), f32, kind="ExternalInput")
    # per-head w_out rows, bf16: [2, 64, 512]
    wo = nc.dram_tensor("wo", (2, D, DIM), bf16, kind="ExternalInput")
    # scales[:, 0:2] = exp(scale_param[h0 + p]); scales[:, 2:4] = -that
    scales = nc.dram_tensor("scales", (P, 4), f32, kind="ExternalInput")
    mk = nc.dram_tensor("mk", (2, NB, P, KNN, D), bf16, kind="ExternalInput")
    mv = nc.dram_tensor("mv", (2, NB, P, KNN, D + 1), bf16, kind="ExternalInput")
    if use_mbias:
        mbias = nc.dram_tensor("mbias", (2, NB, P, KNN), f32, kind="ExternalInput")
    out = nc.dram_tensor("out", (N, DIM), f32, kind="ExternalOutput")

    # constants baked into the NEFF
    eye16_d = nc.inline_tensor(
        np.eye(P, dtype=np.float32).astype(ml_dtypes.bfloat16), name="eye16_c")
    tril_d = nc.inline_tensor(
        np.triu(np.ones((P, P), dtype=np.float32)).astype(ml_dtypes.bfloat16),
        name="tril_c")  # keep j <= q
    e65_np = np.zeros((D + 1, 1), dtype=np.float32)
    e65_np[D, 0] = 1.0
    e65_d = nc.inline_tensor(e65_np.astype(ml_dtypes.bfloat16), name="e65_c")

    with tile.TileContext(nc) as tc:
        with (
            tc.tile_pool(name="singles", bufs=1) as singles,
            tc.tile_pool(name="xin", bufs=2) as xin,
            tc.tile_pool(name="mem", bufs=6) as memp,
            tc.tile_pool(name="mvp", bufs=8) as mvp,
            tc.tile_pool(name="prods", bufs=4) as prods,
            tc.tile_pool(name="small", bufs=12) as small,
            tc.tile_pool(name="pt", bufs=3) as ptp,
            tc.tile_pool(name="stg", bufs=4) as stgp,
            tc.tile_pool(name="outp", bufs=3) as outp,
            tc.tile_pool(name="ppt", bufs=2, space="PSUM") as ppt,
            tc.tile_pool(name="pp512", bufs=2, space="PSUM") as pp512,
            tc.tile_pool(name="ppo", bufs=2, space="PSUM") as ppo,
            tc.tile_pool(name="pden", bufs=2, space="PSUM") as pden,
        ):
            # ---- constants / weights ------------------------------------
            eye16 = singles.tile([P, P], bf16, tag="eye16")
            nc.sync.dma_start(eye16, eye16_d[:, :])
            tril_sb = singles.tile([P, P], bf16, tag="tril")
            nc.sync.dma_start(tril_sb, tril_d[:, :])
            e65 = singles.tile([D + 1, 1], bf16, tag="e65")
            nc.sync.dma_start(e65, e65_d[:, :])
            sc_sb = singles.tile([P, 4], f32, tag="scales")
            nc.sync.dma_start(sc_sb, scales[:, :])
            w_st = singles.tile([P, NCO, 4 * D], f32, tag="wqkv_st")
            nc.sync.dma_start(w_st, wqkv[:, :, :].rearrange("co p c -> p co c"))
            w_sb = singles.tile([P, NCO, 4 * D], f32r, tag="wqkv")
            nc.scalar.copy(out=w_sb, in_=w_st)
            wo_sb = singles.tile([D, 2, DIM], bf16, tag="wo")
            nc.sync.dma_start(wo_sb, wo[:, :, :].rearrange("h d c -> d h c"))

            # ---- xT load (host pre-transposed), 4 n-chunks, round to f32r
            xT = singles.tile([P, NCO, N], f32r, tag="xT")
            for ch in range(4):
                nsl = slice(ch * 512, (ch + 1) * 512)
                x_t = xin.tile([P, NCO, 512], f32, tag="x_in")
                nc.sync.dma_start(x_t, xbT[:, :, nsl].rearrange(
                    "co p n -> p co n"))
                nc.scalar.copy(out=xT[:, :, nsl], in_=x_t)

            # ---- fused q/kv natural projections + sumsq -----------------
            # qkv_sb[:, g, 0:128]  = q for heads p=0,1 (64 cols each)
            # qkv_sb[:, g, 128:192] = k ; 192:256 = v
            qkv_sb = singles.tile([P, NB, 4 * D], f32, tag="qkv")
            ss_all = singles.tile([P, NB, 3], f32, tag="ss")
            rn = singles.tile([P, NB, 3], f32, tag="rn")
            junk = singles.tile([P, D], f32, tag="junk")
            qh = singles.tile([P, 2 * NB, D], bf16, tag="qh")
            kh = singles.tile([P, NB, D], bf16, tag="kh")
            v_aug = singles.tile([P, NB, D + 1], bf16, tag="vaug")
            nc.gpsimd.memset(v_aug[:, :, D:D + 1], 1.0)
            qT = singles.tile([D, 2 * NB, P], bf16, tag="qT")
            kT = singles.tile([D, NB, P], bf16, tag="kT")

            # ---- per-head attention -------------------------------------
            # hoT65[p]: rows 0:64 = un-divided head output (d-space),
            # row 64 = softmax denominator; per g-block of 128 queries.
            hoT65 = [singles.tile([D + 1, NB, P], bf16, tag=f"hoT{p}",
                                  name=f"hoT{p}") for p in range(2)]
            r_all = [singles.tile([P, NB], f32, tag=f"r{p}", name=f"rr{p}")
                     for p in range(2)]
            # block-diagonal stage, zeros written once; one slot per
            # (head, g) so all 32 score chains can run ahead of the PE
            stage4 = singles.tile([P, 2 * NB, 4, KNN], bf16, tag="st4")
            nc.gpsimd.memset(stage4, 0.0)

            def scores_s1(p, g):
                """DVE mem-score front half for block (p, g) -> s_mem f32."""
                idx = p * NB + g
                mk_t = memp.tile([P, KNN, D], bf16, tag="mk")
                nc.sync.dma_start(mk_t, mk[p, g])
                prod = prods.tile([P, KNN, D], bf16, tag="prod")
                nc.vector.tensor_mul(
                    prod, mk_t, qh[:, idx, None, :].to_broadcast((P, KNN, D)))
                s_mem = small.tile([P, KNN], f32, tag="smem")
                if SCORE_TREE == 0:
                    nc.vector.reduce_sum(s_mem, prod, axis=AX.X)
                else:
                    w = D
                    for _ in range(SCORE_TREE):
                        h = w // 2
                        nc.vector.tensor_add(prod[:, :, 0:h], prod[:, :, 0:h],
                                             prod[:, :, h:w])
                        w = h
                    if w > 1:
                        nc.vector.reduce_sum(s_mem, prod[:, :, 0:w], axis=AX.X)
                    else:
                        nc.vector.tensor_copy(out=s_mem, in_=prod[:, :, 0])
                if use_mbias:
                    mb_t = small.tile([P, KNN], f32, tag="mbias")
                    nc.sync.dma_start(mb_t, mbias[p, g])
                    nc.vector.tensor_add(s_mem, s_mem, mb_t)
                return s_mem

            def scores_s2(p, g, s_mem):
                """ACT exp + DVE staging back half (emitted one block behind
                s1 so the DVE never waits on the ACT round-trip)."""
                idx = p * NB + g
                p_mem = small.tile([P, KNN], bf16, tag="pmem")
                nc.scalar.activation(out=p_mem, in_=s_mem, func=ACTF.Exp,
                                     bias=sc_sb[:, 2 + p:3 + p],
                                     scale=sc_sb[:, p:p + 1])
                # 32x32 band-local transpose -> block-diagonal content
                stD = stgp.tile([P, KNN], bf16, tag="stD")
                nc.vector.transpose(stD, p_mem)
                slot = stage4[:, idx, :, :]
                for ql in range(4):
                    nc.vector.tensor_copy(
                        out=slot[32 * ql:32 * (ql + 1), ql, :],
                        in_=stD[32 * ql:32 * (ql + 1), :])
                return slot

            # ---- fused setup pipeline, per g-block ----------------------
            # projection -> norms -> q-hat/k-hat/v -> score chains, all
            # interleaved so every engine starts within ~15us. Score chains
            # are software-pipelined one block deep (DVE never waits on the
            # ACT exp round-trip). qT/kT transposes batch every 4 blocks.
            all_slots = {}
            pend = None
            for g in range(NB):
                qsl = slice(g * P, (g + 1) * P)
                qkv_full = pp512.tile([P, 512], f32, tag="st", name="qkv_ps")
                qkv_ps = qkv_full[:, 0:4 * D]
                for co in range(NCO):
                    nc.tensor.matmul(qkv_ps, xT[:, co, qsl], w_sb[:, co, :],
                                     start=(co == 0), stop=(co == NCO - 1))
                nc.scalar.copy(out=qkv_sb[:, g, :], in_=qkv_ps)
                for c in range(3):
                    nc.scalar.activation(out=junk, in_=qkv_ps[:, c * D:(c + 1) * D],
                                         func=ACTF.Square,
                                         accum_out=ss_all[:, g, c:c + 1])
                nrm_t = small.tile([P, 3], f32, tag="nrm")
                nc.scalar.sqrt(nrm_t, ss_all[:, g, :])
                nc.vector.reciprocal(rn[:, g, :], nrm_t)
                for p in range(2):
                    nc.vector.tensor_scalar_mul(
                        qh[:, p * NB + g, :], qkv_sb[:, g, p * D:(p + 1) * D],
                        rn[:, g, p:p + 1])
                nc.vector.tensor_scalar_mul(kh[:, g, :],
                                            qkv_sb[:, g, 2 * D:3 * D],
                                            rn[:, g, 2:3])
                nc.vector.tensor_copy(out=v_aug[:, g, 0:D],
                                      in_=qkv_sb[:, g, 3 * D:4 * D])
                if g % 4 == 3:
                    g0 = g - 3
                    for pi in range(2):
                        t_ps = ppt.tile([D, 4, P], bf16, tag="tps")
                        for i4 in range(4):
                            nc.tensor.transpose(
                                t_ps[:, i4, :], qh[:, pi * NB + g0 + i4, :],
                                eye16)
                        nc.scalar.copy(
                            out=qT[:, pi * NB + g0:pi * NB + g0 + 4, :],
                            in_=t_ps)
                    t_ps = ppt.tile([D, 4, P], bf16, tag="tps")
                    for i4 in range(4):
                        nc.tensor.transpose(t_ps[:, i4, :], kh[:, g0 + i4, :],
                                            eye16)
                    nc.scalar.copy(out=kT[:, g0:g0 + 4, :], in_=t_ps)

            # all 32 score chains after the setup DVE ops, software-
            # pipelined one block deep against the ACT exp round-trip
            for p in range(2):
                for g in range(NB):
                    s_mem = scores_s1(p, g)
                    if pend is not None:
                        all_slots[(pend[0], pend[1])] = scores_s2(*pend)
                    pend = (p, g, s_mem)
            all_slots[(pend[0], pend[1])] = scores_s2(*pend)

            for p in range(2):
                sc_ap = sc_sb[:, p:p + 1]
                nb_ap = sc_sb[:, 2 + p:3 + p]
                for qc in range(4):
                    g_lo4 = 4 * qc
                    slots = [all_slots[(p, g_lo4 + gi)] for gi in range(4)]
                    psum_oT = ppo.tile([D + 1, 4, P], f32, tag="po")
                    # local attention, [65, q] orientation
                    for jt in range(4 * qc + 4):
                        g_lo = max(jt, g_lo4)
                        ng = g_lo4 + 4 - g_lo
                        i_lo = p * NB + g_lo
                        st_ps = pp512.tile([P, 512], f32, tag="st")
                        nc.tensor.matmul(
                            st_ps[:, :ng * P], kT[:, jt, :],
                            qT[:, i_lo:i_lo + ng, :], start=True, stop=True)
                        p_t = ptp.tile([P, 4, P], bf16, tag="pt")
                        nc.scalar.activation(
                            out=p_t[:, :ng, :],
                            in_=st_ps[:, :ng * P].rearrange(
                                "p (g q) -> p g q", q=P),
                            func=ACTF.Exp, bias=nb_ap, scale=sc_ap)
                        if g_lo <= jt < g_lo4 + 4:
                            di = jt - g_lo
                            nc.gpsimd.tensor_mul(p_t[:, di, :], p_t[:, di, :],
                                                 tril_sb)
                        nc.tensor.matmul(
                            psum_oT[:, g_lo - g_lo4:, :], v_aug[:, jt, :],
                            p_t[:, :ng, :], start=(jt == 0), stop=False)
                    # mem values: accumulate straight into psum_oT
                    for gi in range(4):
                        g = g_lo4 + gi
                        mv_t = mvp.tile([P, KNN, D + 1], bf16, tag="mv")
                        # ACT-engine HWDGE queue: independent of the mk
                        # stream on sync, so pm never waits behind it
                        nc.scalar.dma_start(mv_t, mv[p, g])
                        ovw = psum_oT[:, gi, :].rearrange(
                            "p (ql gf) -> p gf ql", gf=KNN)
                        for g4 in range(KNN):
                            nc.tensor.matmul(
                                ovw[:, g4, :], mv_t[:, g4, :],
                                slots[gi][:, :, g4], start=False,
                                stop=(gi == 3 and g4 == KNN - 1))
                    # un-divided head rows + denominator row -> SBUF
                    nc.scalar.copy(out=hoT65[p][:, g_lo4:g_lo4 + 4, :],
                                   in_=psum_oT)
                # denominators -> q-space + reciprocal (recip lands right
                # after the score chains drain on the DVE)
                den_t = pden.tile([P, NB], f32, tag="den")
                for g in range(NB):
                    nc.tensor.matmul(den_t[:, g:g + 1], hoT65[p][:, g, :],
                                     e65, start=True, stop=True)
                nc.vector.reciprocal(r_all[p], den_t)

            # ---- per-head output projection + divide + combine ----------
            for g in range(NB):
                pf0 = pp512.tile([P, DIM], f32, tag="st", name="pf0")
                nc.tensor.matmul(pf0, hoT65[0][0:D, g, :], wo_sb[:, 0, :],
                                 start=True, stop=True)
                pf1 = pp512.tile([P, DIM], f32, tag="st", name="pf1")
                nc.tensor.matmul(pf1, hoT65[1][0:D, g, :], wo_sb[:, 1, :],
                                 start=True, stop=True)
                of_s = outp.tile([P, DIM], f32, tag="ofs")
                nc.scalar.mul(of_s, pf0, r_all[0][:, g:g + 1])
                nc.vector.scalar_tensor_tensor(
                    out=of_s, in0=pf1, scalar=r_all[1][:, g:g + 1], in1=of_s,
                    op0=MUL, op1=ADD)
                nc.sync.dma_start(out[g * P:(g + 1) * P, :], of_s)

    nc.compile()
    return nc


def _prep_mv(mv_slice):
    """[2,2048,32,64] -> [2,16,128,32,65] bf16: partition (ql k) stacks the 4
    stride-32 queries of each group; col 64 = 1.0 (softmax-denominator)."""
    import ml_dtypes
    r = mv_slice.reshape(2, NB, 4, KNN, KNN, D).transpose(0, 1, 2, 4, 3, 5)
    out = np.empty((2, NB, P, KNN, D + 1), dtype=ml_dtypes.bfloat16)
    out[..., :D] = r.reshape(2, NB, P, KNN, D).astype(ml_dtypes.bfloat16)
    out[..., D] = 1.0
    return out


def _prepare_in_maps(x, w_q, w_kv, w_out, scale_param, mem_k, mem_v, mem_mask,
                     use_mbias):
    import ml_dtypes
    f = np.float32
    bf = ml_dtypes.bfloat16
    scales8 = np.exp(scale_param.reshape(HEADS).astype(f))
    in_maps = []
    for c in range(NCORES):
        b = c // 4
        h0 = 2 * (c % 4)
        sc = np.empty((P, 4), dtype=f)
        sc[:, 0] = scales8[h0]
        sc[:, 1] = scales8[h0 + 1]
        sc[:, 2] = -scales8[h0]
        sc[:, 3] = -scales8[h0 + 1]
        wcat = np.concatenate(
            [w_q[:, h0 * D:(h0 + 2) * D], w_kv], axis=1).astype(f)
        m = {
            "xbT": np.ascontiguousarray(
                x[b].T.reshape(NCO, P, N), dtype=f),
            "wqkv": np.ascontiguousarray(wcat.reshape(NCO, P, 4 * D)),
            "wo": np.ascontiguousarray(
                w_out[h0 * D:(h0 + 2) * D, :].reshape(2, D, DIM).astype(bf)),
            "scales": sc,
            "mk": np.ascontiguousarray(
                mem_k[b, h0:h0 + 2].reshape(2, NB, P, KNN, D).astype(bf)),
            "mv": _prep_mv(mem_v[b, h0:h0 + 2]),
        }
        if use_mbias:
            mb = np.where(mem_mask[b, h0:h0 + 2], f(0), f(-1e30)).astype(f)
            m["mbias"] = np.ascontiguousarray(mb.reshape(2, NB, P, KNN))
        in_maps.append(m)
    return in_maps


def _run(x, w_q, w_kv, w_out, scale_param, mem_k, mem_v, mem_mask, trace=False):
    from concourse.bass_utils import run_bass_kernel_spmd

    use_mbias = not bool(np.all(mem_mask))
    nc = _build(use_mbias)
    in_maps = _prepare_in_maps(x, w_q, w_kv, w_out, scale_param,
                               mem_k, mem_v, mem_mask, use_mbias)
    res = run_bass_kernel_spmd(nc, in_maps, core_ids=list(range(NCORES)),
                               trace=trace)
    out = np.zeros((B, N, DIM), dtype=np.float32)
    for c in range(NCORES):
        out[c // 4] += res.results[c]["out"]
    return out, res


def kernel(x, w_q, w_kv, w_out, scale_param, mem_k, mem_v, mem_mask):
    trace = bool(int(os.environ.get("BASS_KERNEL_TRACE", "0")))
    out, _ = _run(x, w_q, w_kv, w_out, scale_param, mem_k, mem_v, mem_mask,
                  trace=trace)
    return out
